# revision 7
# baseline (speedup 1.0000x reference)
"""DGCNN (B=8 point clouds, N=4096 points, K=5 NN graph) on 8 Trainium2 cores.

Sharding: data-parallel, one point cloud per NeuronCore. Two device phases:
  L1: per-point MLP (3->64->64->64->128) + pairwise -d2 matmul + top-8
      nearest-neighbor search (DVE max8/max_index) per node.
  host: degree histogram + rsqrt norms + gather-index packing (tiny numpy).
  L2: 3 GCN layers (gather-sum over 5 neighbors via SWDGE dma_gather,
      matmuls on PE, per-node norm scaling) + MLP head + per-core max-pool.
  host: 8-way max reduce + final 256->128 matvec.
"""

import time

import numpy as np

import concourse.bacc as bacc
import concourse.mybir as mybir
import concourse.bass as bass
from concourse.tile import TileContext

dt = mybir.dt
AF = mybir.ActivationFunctionType

B, N, K = 8, 4096, 5
NT = N // 128          # 32 node tiles
NE = N * K             # 20480 edges per cloud
GCH = 8                # gather chunks per layer
GIDX = NE // GCH       # 2560 indices per gather chunk
F32 = dt.float32


def _build_l1():
    nc = bacc.Bacc(name="dgcnn_l1")
    xs_d = nc.dram_tensor("xs", [3, N], F32, kind="ExternalInput")
    lhs5_d = nc.dram_tensor("lhs5", [5, N], F32, kind="ExternalInput")
    rhs5_d = nc.dram_tensor("rhs5", [5, N], F32, kind="ExternalInput")
    w1_d = nc.dram_tensor("w1", [3, 64], F32, kind="ExternalInput")
    w2_d = nc.dram_tensor("w2", [64, 64], F32, kind="ExternalInput")
    w3_d = nc.dram_tensor("w3", [64, 64], F32, kind="ExternalInput")
    w4_d = nc.dram_tensor("w4", [64, 128], F32, kind="ExternalInput")
    b1_d = nc.dram_tensor("b1", [64, 1], F32, kind="ExternalInput")
    b2_d = nc.dram_tensor("b2", [64, 1], F32, kind="ExternalInput")
    b3_d = nc.dram_tensor("b3", [64, 1], F32, kind="ExternalInput")
    b4_d = nc.dram_tensor("b4", [128, 1], F32, kind="ExternalInput")
    feat_d = nc.dram_tensor("feat", [128, N], F32, kind="ExternalOutput")
    nbr_d = nc.dram_tensor("nbr", [N, 8], dt.uint32, kind="ExternalOutput")

    with TileContext(nc) as tc:
        with (
            tc.tile_pool(name="const", bufs=1) as cp,
            tc.tile_pool(name="act", bufs=2) as ap,
            tc.tile_pool(name="d2", bufs=3) as dp,
            tc.tile_pool(name="knn", bufs=3) as kp,
            tc.tile_pool(name="cps", bufs=2, space="PSUM") as cpp,
            tc.tile_pool(name="dps", bufs=4, space="PSUM") as pp,
        ):
            lhs5 = cp.tile([5, N], F32)
            rhs5 = cp.tile([5, N], F32)
            nc.sync.dma_start(out=lhs5[:], in_=lhs5_d[:])
            nc.sync.dma_start(out=rhs5[:], in_=rhs5_d[:])
            ws, bs = [], []
            for wd, bd, shp in ((w1_d, b1_d, [3, 64]), (w2_d, b2_d, [64, 64]),
                                (w3_d, b3_d, [64, 64]), (w4_d, b4_d, [64, 128])):
                w = cp.tile(shp, F32, tag=f"w{len(ws)}")
                b = cp.tile([shp[1], 1], F32, tag=f"b{len(bs)}")
                nc.sync.dma_start(out=w[:], in_=wd[:])
                nc.sync.dma_start(out=b[:], in_=bd[:])
                ws.append(w)
                bs.append(b)

            x0 = ap.tile([3, N], F32, tag="x0")
            nc.sync.dma_start(out=x0[:], in_=xs_d[:])

            # conv stack, feature-major, weights-stationary
            h = x0
            for li, (w, b) in enumerate(zip(ws, bs)):
                fo = w.shape[1]
                hn = ap.tile([fo, N], F32, tag="conv")
                for j in range(N // 512):
                    ps = cpp.tile([fo, 512], F32, space="PSUM", tag="convps")
                    nc.tensor.matmul(out=ps[:], lhsT=w[:], rhs=h[:, j * 512:(j + 1) * 512],
                                     start=True, stop=True)
                    nc.scalar.activation(out=hn[:, j * 512:(j + 1) * 512], in_=ps[:],
                                         func=AF.Relu, bias=b[:, :1])
                h = hn
            nc.sync.dma_start(out=feat_d[:], in_=h[:])

            # -d2 tiles + top8
            for t in range(NT):
                d2sb = dp.tile([128, N], F32, tag="d2sb")
                for j in range(N // 512):
                    ps = pp.tile([128, 512], F32, space="PSUM", tag="d2ps")
                    nc.tensor.matmul(out=ps[:], lhsT=lhs5[:, t * 128:(t + 1) * 128],
                                     rhs=rhs5[:, j * 512:(j + 1) * 512], start=True, stop=True)
                    nc.scalar.activation(out=d2sb[:, j * 512:(j + 1) * 512], in_=ps[:],
                                         func=AF.Copy)
                tv = kp.tile([128, 8], F32, tag="tv")
                ti = kp.tile([128, 8], dt.uint32, tag="ti")
                nc.vector.max(tv[:], d2sb[:])
                nc.vector.max_index(ti[:], tv[:], d2sb[:])
                nc.sync.dma_start(out=nbr_d[t * 128:(t + 1) * 128, :], in_=ti[:])
    nc.finalize()
    return nc


def _build_l2():
    nc = bacc.Bacc(name="dgcnn_l2")
    feat_d = nc.dram_tensor("feat", [128, N], F32, kind="ExternalInput")
    ident_d = nc.dram_tensor("ident", [128, 128], F32, kind="ExternalInput")
    gidx_d = nc.dram_tensor("gidx", [128, NE // 128], dt.int32, kind="ExternalInput")
    nout_d = nc.dram_tensor("nout", [128, NT], F32, kind="ExternalInput")
    g1w_d = nc.dram_tensor("g1w", [128, 128], F32, kind="ExternalInput")
    g2w_d = nc.dram_tensor("g2w", [128, 128], F32, kind="ExternalInput")
    g3w_d = nc.dram_tensor("g3w", [128, 256], F32, kind="ExternalInput")
    g1b_d = nc.dram_tensor("g1b", [128, 1], F32, kind="ExternalInput")
    g2b_d = nc.dram_tensor("g2b", [128, 1], F32, kind="ExternalInput")
    w1t_d = nc.dram_tensor("w1t", [256, 512], F32, kind="ExternalInput")
    b1_d = nc.dram_tensor("b1", [512, 1], F32, kind="ExternalInput")
    w2t_d = nc.dram_tensor("w2t", [512, 256], F32, kind="ExternalInput")
    b2_d = nc.dram_tensor("b2", [256, 1], F32, kind="ExternalInput")
    pooled_d = nc.dram_tensor("pooled", [256, 1], F32, kind="ExternalOutput")

    with TileContext(nc) as tc:
        with (
            tc.tile_pool(name="const", bufs=1) as cp,
            tc.tile_pool(name="act", bufs=2) as ap,
            tc.tile_pool(name="nm", bufs=4) as nmp,
            tc.tile_pool(name="gth", bufs=6) as gp,
            tc.tile_pool(name="hd", bufs=8) as hp,
            tc.tile_pool(name="ps1", bufs=2, space="PSUM") as pp1,
            tc.tile_pool(name="ps2", bufs=2, space="PSUM") as pp2,
            tc.tile_pool(name="dram", bufs=2, space="DRAM") as drp,
        ):
            ident = cp.tile([128, 128], F32)
            nc.sync.dma_start(out=ident[:], in_=ident_d[:])
            gidx = cp.tile([128, NE // 128], dt.int32)
            nc.sync.dma_start(out=gidx[:], in_=gidx_d[:])
            nout = cp.tile([128, NT], F32)
            nc.sync.dma_start(out=nout[:], in_=nout_d[:])
            g1w = cp.tile([128, 128], F32, tag="g1w")
            g2w = cp.tile([128, 128], F32, tag="g2w")
            g3w = cp.tile([128, 256], F32, tag="g3w")
            g1b = cp.tile([128, 1], F32, tag="g1b")
            g2b = cp.tile([128, 1], F32, tag="g2b")
            for t_, d_ in ((g1w, g1w_d), (g2w, g2w_d), (g3w, g3w_d), (g1b, g1b_d), (g2b, g2b_d)):
                nc.sync.dma_start(out=t_[:], in_=d_[:])
            w1t = [cp.tile([128, 512], F32, tag=f"w1t{i}", name=f"w1t{i}") for i in range(2)]
            for i in range(2):
                nc.sync.dma_start(out=w1t[i][:], in_=w1t_d[i * 128:(i + 1) * 128, :])
            w2t = [cp.tile([128, 256], F32, tag=f"w2t{i}", name=f"w2t{i}") for i in range(4)]
            for i in range(4):
                nc.sync.dma_start(out=w2t[i][:], in_=w2t_d[i * 128:(i + 1) * 128, :])
            b1 = [cp.tile([128, 1], F32, tag=f"b1_{i}", name=f"b1_{i}") for i in range(4)]
            for i in range(4):
                nc.sync.dma_start(out=b1[i][:], in_=b1_d[i * 128:(i + 1) * 128, :])
            b2 = [cp.tile([128, 1], F32, tag=f"b2_{i}", name=f"b2_{i}") for i in range(2)]
            for i in range(2):
                nc.sync.dma_start(out=b2[i][:], in_=b2_d[i * 128:(i + 1) * 128, :])

            h = ap.tile([128, N], F32, tag="act")
            nc.sync.dma_start(out=h[:], in_=feat_d[:])

            def gcn_gather_layer(h_in, w_rhs, bias, relu, out_tag):
                """v2 layer: node-tile-stationary matmul -> nm scale -> DRAM ->
                gather-sum -> transpose -> ACT epilogue -> feature-major out."""
                hw_t = drp.tile([N, 128], F32, tag="hwtbl")
                # matmul + norm_out scale + store node-major rows to DRAM
                for t in range(NT):
                    psA = pp1.tile([128, 128], F32, space="PSUM", tag="psA")
                    nc.tensor.matmul(out=psA[:], lhsT=h_in[:, t * 128:(t + 1) * 128],
                                     rhs=w_rhs[:], start=True, stop=True)
                    hw_nm = nmp.tile([128, 128], F32, tag="hw_nm")
                    nc.scalar.activation(out=hw_nm[:], in_=psA[:], func=AF.Copy,
                                         scale=nout[:, t:t + 1])
                    nc.sync.dma_start(out=hw_t[t * 128:(t + 1) * 128, :], in_=hw_nm[:])
                # gather + 5-sum + transpose + epilogue
                h_out = ap.tile([128, N], F32, tag=out_tag)
                for t in range(NT):
                    gath = gp.tile([128, K, 128], F32, tag="gath")
                    for k in range(K):
                        nc.gpsimd.indirect_dma_start(
                            out=gath[:, k, :], out_offset=None, in_=hw_t[:],
                            in_offset=bass.IndirectOffsetOnAxis(ap=gidx[:, t * K + k:t * K + k + 1], axis=0))
                    agg = nmp.tile([128, 128], F32, tag="agg")
                    nc.vector.tensor_reduce(
                        out=agg[:], in_=gath[:].rearrange("p k f -> p f k"),
                        axis=mybir.AxisListType.X, op=mybir.AluOpType.add)
                    psT = pp2.tile([128, 128], F32, space="PSUM", tag="psT")
                    nc.tensor.transpose(out=psT[:], in_=agg[:], identity=ident[:])
                    nc.scalar.activation(out=h_out[:, t * 128:(t + 1) * 128], in_=psT[:],
                                         func=AF.Relu if relu else AF.Copy,
                                         bias=bias[:, :1] if bias is not None else 0.0)
                return h_out

            h1 = gcn_gather_layer(h, g1w, g1b, True, "act")
            h2 = gcn_gather_layer(h1, g2w, g2b, True, "act")

            # gc3: aggregate-first. scale+transpose h2 -> node-major DRAM
            hw_t = drp.tile([N, 128], F32, tag="hwtbl")
            for t in range(NT):
                psT = pp2.tile([128, 128], F32, space="PSUM", tag="psT")
                nc.tensor.transpose(out=psT[:], in_=h2[:, t * 128:(t + 1) * 128], identity=ident[:])
                tmp_nm = nmp.tile([128, 128], F32, tag="hw_nm")
                nc.scalar.activation(out=tmp_nm[:], in_=psT[:], func=AF.Copy,
                                     scale=nout[:, t:t + 1])
                nc.sync.dma_start(out=hw_t[t * 128:(t + 1) * 128, :], in_=tmp_nm[:])
            agg3 = ap.tile([128, N], F32, tag="act")
            for t in range(NT):
                gath = gp.tile([128, K, 128], F32, tag="gath")
                for k in range(K):
                    nc.gpsimd.indirect_dma_start(
                        out=gath[:, k, :], out_offset=None, in_=hw_t[:],
                        in_offset=bass.IndirectOffsetOnAxis(ap=gidx[:, t * K + k:t * K + k + 1], axis=0))
                agg = nmp.tile([128, 128], F32, tag="agg")
                nc.vector.tensor_reduce(
                    out=agg[:], in_=gath[:].rearrange("p k f -> p f k"),
                    axis=mybir.AxisListType.X, op=mybir.AluOpType.add)
                psT = pp2.tile([128, 128], F32, space="PSUM", tag="psT")
                nc.tensor.transpose(out=psT[:], in_=agg[:], identity=ident[:])
                nc.scalar.activation(out=agg3[:, t * 128:(t + 1) * 128], in_=psT[:], func=AF.Copy)
            # g3 matmul: [256, N] feature-major, weights-stationary (2 col-blocks)
            g3 = [ap.tile([128, N], F32, tag="g3", name=f"g3_{i}") for i in range(2)]
            for half in range(2):
                for j in range(N // 512):
                    ps = pp1.tile([128, 512], F32, space="PSUM", tag="psB")
                    nc.tensor.matmul(out=ps[:], lhsT=g3w[:, half * 128:(half + 1) * 128],
                                     rhs=agg3[:, j * 512:(j + 1) * 512], start=True, stop=True)
                    nc.scalar.activation(out=g3[half][:, j * 512:(j + 1) * 512], in_=ps[:], func=AF.Copy)

            # head, streamed in 512-col chunks
            poolacc = cp.tile([128, 2], F32, tag="poolacc")
            first = [True, True]
            for j in range(N // 512):
                z1 = [hp.tile([128, 512], F32, tag="z1", name=f"z1_{i}") for i in range(4)]
                for mi in range(4):
                    ps = pp1.tile([128, 512], F32, space="PSUM", tag="psB")
                    for ki in range(2):
                        nc.tensor.matmul(out=ps[:], lhsT=w1t[ki][:, mi * 128:(mi + 1) * 128],
                                         rhs=g3[ki][:, j * 512:(j + 1) * 512],
                                         start=(ki == 0), stop=(ki == 1))
                    nc.scalar.activation(out=z1[mi][:], in_=ps[:], func=AF.Relu, bias=b1[mi][:, :1])
                for mi in range(2):
                    ps = pp1.tile([128, 512], F32, space="PSUM", tag="psB")
                    for ki in range(4):
                        nc.tensor.matmul(out=ps[:], lhsT=w2t[ki][:, mi * 128:(mi + 1) * 128],
                                         rhs=z1[ki][:, :], start=(ki == 0), stop=(ki == 3))
                    z2 = hp.tile([128, 512], F32, tag="z2")
                    nc.scalar.activation(out=z2[:], in_=ps[:], func=AF.Relu, bias=b2[mi][:, :1])
                    cmax = hp.tile([128, 1], F32, tag="cmax")
                    nc.vector.tensor_reduce(out=cmax[:], in_=z2[:], axis=mybir.AxisListType.X,
                                            op=mybir.AluOpType.max)
                    if first[mi]:
                        nc.vector.tensor_copy(out=poolacc[:, mi:mi + 1], in_=cmax[:])
                        first[mi] = False
                    else:
                        nc.vector.tensor_tensor(out=poolacc[:, mi:mi + 1], in0=poolacc[:, mi:mi + 1],
                                                in1=cmax[:], op=mybir.AluOpType.max)
            nc.sync.dma_start(out=pooled_d[0:128, :], in_=poolacc[:, 0:1])
            nc.sync.dma_start(out=pooled_d[128:256, :], in_=poolacc[:, 1:2])
    nc.finalize()
    return nc


def _make_runner(nc, n_cores=8):
    import jax
    from jax.sharding import Mesh, PartitionSpec
    from jax.experimental.shard_map import shard_map
    from concourse import bass2jax

    partition_name = nc.partition_id_tensor.name
    in_names, out_names, out_avals, zero_outs = [], [], [], []
    for alloc in nc.m.functions[0].allocations:
        if not isinstance(alloc, mybir.MemoryLocationSet):
            continue
        name = alloc.memorylocations[0].name
        if alloc.kind == "ExternalInput":
            if name != partition_name:
                in_names.append(name)
        elif alloc.kind == "ExternalOutput":
            shape = tuple(alloc.tensor_shape)
            dtype = dt.np(alloc.dtype)
            out_names.append(name)
            out_avals.append(jax.core.ShapedArray(shape, dtype))
            zero_outs.append(np.zeros(shape, dtype))
    n_params = len(in_names)
    all_in_names = in_names + out_names + [partition_name]

    def _body(*args):
        operands = list(args) + [bass2jax.partition_id_tensor()]
        outs = bass2jax._bass_exec_p.bind(
            *operands, out_avals=tuple(out_avals), in_names=tuple(all_in_names),
            out_names=tuple(out_names), lowering_input_output_aliases=(),
            sim_require_finite=True, sim_require_nnan=True, nc=nc)
        return tuple(outs)

    mesh = Mesh(np.asarray(jax.devices()[:n_cores]), ("core",))
    donate = tuple(range(n_params, n_params + len(out_names)))
    sharded = jax.jit(
        shard_map(_body, mesh=mesh,
                  in_specs=(PartitionSpec("core"),) * (n_params + len(out_names)),
                  out_specs=(PartitionSpec("core"),) * len(out_names), check_rep=False),
        donate_argnums=donate, keep_unused=True)

    def run(in_maps, timeit=0):
        import jax as _jax
        concat_in = [np.concatenate([np.asarray(in_maps[c][n]) for c in range(n_cores)], axis=0)
                     for n in in_names]

        def one_call():
            cz = [np.zeros((n_cores * z.shape[0], *z.shape[1:]), z.dtype) for z in zero_outs]
            return _jax.block_until_ready(sharded(*concat_in, *cz))

        outs = one_call()
        results = [{n: np.asarray(outs[i]).reshape(n_cores, *out_avals[i].shape)[c]
                    for i, n in enumerate(out_names)} for c in range(n_cores)]
        tmin = None
        if timeit:
            times = []
            for _ in range(timeit):
                t0 = time.perf_counter()
                one_call()
                times.append(time.perf_counter() - t0)
            tmin = min(times)
        return results, tmin
    return run


def _build_null():
    nc = bacc.Bacc(name="null")
    x_d = nc.dram_tensor("x", [128, 8], F32, kind="ExternalInput")
    y_d = nc.dram_tensor("y", [128, 8], F32, kind="ExternalOutput")
    with TileContext(nc) as tc:
        with tc.tile_pool(name="p", bufs=1) as p:
            t = p.tile([128, 8], F32)
            nc.sync.dma_start(out=t[:], in_=x_d[:])
            nc.sync.dma_start(out=y_d[:], in_=t[:])
    nc.finalize()
    return nc


_CACHE = {}


def _get_programs():
    if "l1" not in _CACHE:
        _CACHE["l1"] = _make_runner(_build_l1())
        _CACHE["l2"] = _make_runner(_build_l2())
    return _CACHE["l1"], _CACHE["l2"]


def kernel(pointcloud, conv_w1, bn1_g, bn1_b, conv_w2, bn2_g, bn2_b,
           conv_w3, bn3_g, bn3_b, conv_w4, bn4_g, bn4_b,
           gc1_w, gc1_b, gc2_w, gc2_b, gc3_w, gc3_b,
           fc1_w, bnf1_g, bnf1_b, fc2_w, bnf2_g, bnf2_b, fc3_w, fc3_b):
    f32 = np.float32
    pts = np.asarray(pointcloud, f32)[..., 0]              # [B, 3, N]
    centroid = pts.mean(axis=2, keepdims=True, dtype=f32)
    d_cent = ((pts - centroid) ** 2).sum(axis=1, dtype=f32)
    order = np.argsort(d_cent, axis=-1, kind="stable")
    xs = np.take_along_axis(pts, order[:, None, :], axis=2)  # [B, 3, N]

    sq = (pts ** 2).sum(axis=1, dtype=f32)                  # [B, N]
    lhs5 = np.zeros((B, 5, N), f32)
    lhs5[:, 0:3] = pts
    lhs5[:, 3] = -sq
    lhs5[:, 4] = -1.0
    rhs5 = np.zeros((B, 5, N), f32)
    rhs5[:, 0:3] = 2.0 * pts
    rhs5[:, 3] = 1.0
    rhs5[:, 4] = sq

    def effw(w, g):
        return (np.asarray(g, f32)[:, None] * np.asarray(w, f32)).T.copy()  # [fi, fo]

    w1, w2 = effw(conv_w1, bn1_g), effw(conv_w2, bn2_g)
    w3, w4 = effw(conv_w3, bn3_g), effw(conv_w4, bn4_g)
    b14 = [np.asarray(b, f32).reshape(-1, 1) for b in (bn1_b, bn2_b, bn3_b, bn4_b)]

    run1, run2 = _get_programs()
    ins1 = [{
        "xs": np.ascontiguousarray(xs[c]), "lhs5": np.ascontiguousarray(lhs5[c]),
        "rhs5": np.ascontiguousarray(rhs5[c]),
        "w1": w1, "w2": w2, "w3": w3, "w4": w4,
        "b1": b14[0], "b2": b14[1], "b3": b14[2], "b4": b14[3],
    } for c in range(B)]
    _CACHE["ins1"] = ins1
    res1, _ = run1(ins1)

    nrm5 = f32(1.0) / np.sqrt(f32(5.0))
    g1w = np.asarray(gc1_w, f32) * nrm5                    # [128, 128] fi x fo
    g2w = np.asarray(gc2_w, f32) * nrm5
    g3w = np.asarray(gc3_w, f32) * nrm5                    # [128, 256]
    w1_eff = np.asarray(bnf1_g, f32)[:, None] * np.asarray(fc1_w, f32)   # [512, 256]
    b1_eff = (np.asarray(bnf1_b, f32)
              + np.asarray(bnf1_g, f32) * (np.asarray(fc1_w, f32) @ np.asarray(gc3_b, f32)))
    w2_eff = np.asarray(bnf2_g, f32)[:, None] * np.asarray(fc2_w, f32)   # [256, 512]
    b2_eff = np.asarray(bnf2_b, f32)

    ins2 = []
    for c in range(B):
        nbr8 = res1[c]["nbr"]                      # [N, 8] uint32
        nbr5 = nbr8[:, :K].astype(np.int64)
        deg = np.bincount(nbr5.reshape(-1), minlength=N).astype(f32)
        norm_out = (f32(1.0) / np.sqrt(np.maximum(deg, f32(1.0)))).astype(f32)
        gidx = nbr5.reshape(NT, 128, K).transpose(1, 0, 2).reshape(128, -1).astype(np.int32).copy()
        ins2.append({
            "feat": res1[c]["feat"],
            "ident": np.eye(128, dtype=f32),
            "gidx": gidx,
            "nout": np.ascontiguousarray(norm_out.reshape(NT, 128).T),
            "g1w": g1w, "g2w": g2w, "g3w": g3w,
            "g1b": np.asarray(gc1_b, f32).reshape(-1, 1),
            "g2b": np.asarray(gc2_b, f32).reshape(-1, 1),
            "w1t": np.ascontiguousarray(w1_eff.T), "b1": b1_eff.reshape(-1, 1),
            "w2t": np.ascontiguousarray(w2_eff.T), "b2": b2_eff.reshape(-1, 1),
        })
    _CACHE["ins2"] = ins2
    res2, _ = run2(ins2)

    pooled = np.max(np.stack([res2[c]["pooled"][:, 0] for c in range(B)]), axis=0)
    out = np.asarray(fc3_w, f32) @ pooled + np.asarray(fc3_b, f32)
    return out.astype(f32)


def measure_hw_ns(reps=30):
    """Estimate device exec time (ns) of L1+L2 via wall-min minus null overhead.

    Must be called after kernel() so the phase inputs are cached.
    """
    run1, run2 = _get_programs()
    if "null" not in _CACHE:
        _CACHE["null"] = _make_runner(_build_null())
    nullrun = _CACHE["null"]
    nins = [{"x": np.zeros((128, 8), np.float32)}] * 8
    _, t0 = nullrun(nins, timeit=reps)
    _, t1 = run1(_CACHE["ins1"], timeit=reps)
    _, t2 = run2(_CACHE["ins2"], timeit=reps)
    l1_ns = max(0.0, (t1 - t0)) * 1e9
    l2_ns = max(0.0, (t2 - t0)) * 1e9
    return l1_ns, l2_ns, t0 * 1e9


# revision 9
# speedup vs baseline: 814.8798x; 814.8798x over previous
"""DGCNN (B=8 point clouds, N=4096 points, K=5 NN graph) on 8 Trainium2 cores.

Sharding: data-parallel, one point cloud per NeuronCore. Two device phases:
  L1: per-point MLP (3->64->64->64->128) + pairwise -d2 matmul + top-8
      nearest-neighbor search (DVE max8/max_index) per node.
  host: degree histogram + rsqrt norms + gather-index packing (tiny numpy).
  L2: 3 GCN layers (gather-sum over 5 neighbors via SWDGE dma_gather,
      matmuls on PE, per-node norm scaling) + MLP head + per-core max-pool.
  host: 8-way max reduce + final 256->128 matvec.
"""

import time

import numpy as np

import concourse.bacc as bacc
import concourse.mybir as mybir
import concourse.bass as bass
from concourse.tile import TileContext

dt = mybir.dt
AF = mybir.ActivationFunctionType

B, N, K = 8, 4096, 5
NT = N // 128          # 32 node tiles
NE = N * K             # 20480 edges per cloud
GCH = 8                # gather chunks per layer
GIDX = NE // GCH       # 2560 indices per gather chunk
F32 = dt.float32


def _build_l1():
    nc = bacc.Bacc(name="dgcnn_l1")
    xs_d = nc.dram_tensor("xs", [3, N], F32, kind="ExternalInput")
    lhs5_d = nc.dram_tensor("lhs5", [5, N], F32, kind="ExternalInput")
    rhs5_d = nc.dram_tensor("rhs5", [5, N], F32, kind="ExternalInput")
    w1_d = nc.dram_tensor("w1", [3, 64], F32, kind="ExternalInput")
    w2_d = nc.dram_tensor("w2", [64, 64], F32, kind="ExternalInput")
    w3_d = nc.dram_tensor("w3", [64, 64], F32, kind="ExternalInput")
    w4_d = nc.dram_tensor("w4", [64, 128], F32, kind="ExternalInput")
    b1_d = nc.dram_tensor("b1", [64, 1], F32, kind="ExternalInput")
    b2_d = nc.dram_tensor("b2", [64, 1], F32, kind="ExternalInput")
    b3_d = nc.dram_tensor("b3", [64, 1], F32, kind="ExternalInput")
    b4_d = nc.dram_tensor("b4", [128, 1], F32, kind="ExternalInput")
    feat_d = nc.dram_tensor("feat", [128, N], F32, kind="ExternalOutput")
    nbr_d = nc.dram_tensor("nbr", [N, 8], dt.uint32, kind="ExternalOutput")

    with TileContext(nc) as tc:
        with (
            tc.tile_pool(name="const", bufs=1) as cp,
            tc.tile_pool(name="act", bufs=2) as ap,
            tc.tile_pool(name="d2", bufs=3) as dp,
            tc.tile_pool(name="knn", bufs=3) as kp,
            tc.tile_pool(name="cps", bufs=2, space="PSUM") as cpp,
            tc.tile_pool(name="dps", bufs=4, space="PSUM") as pp,
        ):
            lhs5 = cp.tile([5, N], F32)
            rhs5 = cp.tile([5, N], F32)
            nc.sync.dma_start(out=lhs5[:], in_=lhs5_d[:])
            nc.sync.dma_start(out=rhs5[:], in_=rhs5_d[:])
            ws, bs = [], []
            for wd, bd, shp in ((w1_d, b1_d, [3, 64]), (w2_d, b2_d, [64, 64]),
                                (w3_d, b3_d, [64, 64]), (w4_d, b4_d, [64, 128])):
                w = cp.tile(shp, F32, tag=f"w{len(ws)}")
                b = cp.tile([shp[1], 1], F32, tag=f"b{len(bs)}")
                nc.sync.dma_start(out=w[:], in_=wd[:])
                nc.sync.dma_start(out=b[:], in_=bd[:])
                ws.append(w)
                bs.append(b)

            x0 = ap.tile([3, N], F32, tag="x0")
            nc.sync.dma_start(out=x0[:], in_=xs_d[:])

            # conv stack, feature-major, weights-stationary
            h = x0
            for li, (w, b) in enumerate(zip(ws, bs)):
                fo = w.shape[1]
                hn = ap.tile([fo, N], F32, tag="conv")
                for j in range(N // 512):
                    ps = cpp.tile([fo, 512], F32, space="PSUM", tag="convps")
                    nc.tensor.matmul(out=ps[:], lhsT=w[:], rhs=h[:, j * 512:(j + 1) * 512],
                                     start=True, stop=True)
                    nc.scalar.activation(out=hn[:, j * 512:(j + 1) * 512], in_=ps[:],
                                         func=AF.Relu, bias=b[:, :1])
                h = hn
            nc.sync.dma_start(out=feat_d[:], in_=h[:])

            # -d2 tiles + top8
            for t in range(NT):
                d2sb = dp.tile([128, N], F32, tag="d2sb")
                for j in range(N // 512):
                    ps = pp.tile([128, 512], F32, space="PSUM", tag="d2ps")
                    nc.tensor.matmul(out=ps[:], lhsT=lhs5[:, t * 128:(t + 1) * 128],
                                     rhs=rhs5[:, j * 512:(j + 1) * 512], start=True, stop=True)
                    nc.scalar.activation(out=d2sb[:, j * 512:(j + 1) * 512], in_=ps[:],
                                         func=AF.Copy)
                tv = kp.tile([128, 8], F32, tag="tv")
                ti = kp.tile([128, 8], dt.uint32, tag="ti")
                nc.vector.max(tv[:], d2sb[:])
                nc.vector.max_index(ti[:], tv[:], d2sb[:])
                nc.sync.dma_start(out=nbr_d[t * 128:(t + 1) * 128, :], in_=ti[:])
    nc.finalize()
    return nc


def _build_l2():
    nc = bacc.Bacc(name="dgcnn_l2")
    feat_d = nc.dram_tensor("feat", [128, N], F32, kind="ExternalInput")
    ident_d = nc.dram_tensor("ident", [128, 128], F32, kind="ExternalInput")
    gidx_d = nc.dram_tensor("gidx", [128, NE // 128], dt.int32, kind="ExternalInput")
    nout_d = nc.dram_tensor("nout", [128, NT], F32, kind="ExternalInput")
    g1w_d = nc.dram_tensor("g1w", [128, 128], F32, kind="ExternalInput")
    g2w_d = nc.dram_tensor("g2w", [128, 128], F32, kind="ExternalInput")
    g3w_d = nc.dram_tensor("g3w", [128, 256], F32, kind="ExternalInput")
    g1b_d = nc.dram_tensor("g1b", [128, 1], F32, kind="ExternalInput")
    g2b_d = nc.dram_tensor("g2b", [128, 1], F32, kind="ExternalInput")
    w1t_d = nc.dram_tensor("w1t", [256, 512], F32, kind="ExternalInput")
    b1_d = nc.dram_tensor("b1", [512, 1], F32, kind="ExternalInput")
    w2t_d = nc.dram_tensor("w2t", [512, 256], F32, kind="ExternalInput")
    b2_d = nc.dram_tensor("b2", [256, 1], F32, kind="ExternalInput")
    pooled_d = nc.dram_tensor("pooled", [256, 1], F32, kind="ExternalOutput")

    with TileContext(nc) as tc:
        with (
            tc.tile_pool(name="const", bufs=1) as cp,
            tc.tile_pool(name="act", bufs=2) as ap,
            tc.tile_pool(name="nm", bufs=4) as nmp,
            tc.tile_pool(name="gth", bufs=6) as gp,
            tc.tile_pool(name="hd", bufs=8) as hp,
            tc.tile_pool(name="ps1", bufs=2, space="PSUM") as pp1,
            tc.tile_pool(name="ps2", bufs=2, space="PSUM") as pp2,
            tc.tile_pool(name="dram", bufs=2, space="DRAM") as drp,
        ):
            ident = cp.tile([128, 128], F32)
            nc.sync.dma_start(out=ident[:], in_=ident_d[:])
            gidx = cp.tile([128, NE // 128], dt.int32)
            nc.sync.dma_start(out=gidx[:], in_=gidx_d[:])
            nout = cp.tile([128, NT], F32)
            nc.sync.dma_start(out=nout[:], in_=nout_d[:])
            g1w = cp.tile([128, 128], F32, tag="g1w")
            g2w = cp.tile([128, 128], F32, tag="g2w")
            g3w = cp.tile([128, 256], F32, tag="g3w")
            g1b = cp.tile([128, 1], F32, tag="g1b")
            g2b = cp.tile([128, 1], F32, tag="g2b")
            for t_, d_ in ((g1w, g1w_d), (g2w, g2w_d), (g3w, g3w_d), (g1b, g1b_d), (g2b, g2b_d)):
                nc.sync.dma_start(out=t_[:], in_=d_[:])
            w1t = [cp.tile([128, 512], F32, tag=f"w1t{i}", name=f"w1t{i}") for i in range(2)]
            for i in range(2):
                nc.sync.dma_start(out=w1t[i][:], in_=w1t_d[i * 128:(i + 1) * 128, :])
            w2t = [cp.tile([128, 256], F32, tag=f"w2t{i}", name=f"w2t{i}") for i in range(4)]
            for i in range(4):
                nc.sync.dma_start(out=w2t[i][:], in_=w2t_d[i * 128:(i + 1) * 128, :])
            b1 = [cp.tile([128, 1], F32, tag=f"b1_{i}", name=f"b1_{i}") for i in range(4)]
            for i in range(4):
                nc.sync.dma_start(out=b1[i][:], in_=b1_d[i * 128:(i + 1) * 128, :])
            b2 = [cp.tile([128, 1], F32, tag=f"b2_{i}", name=f"b2_{i}") for i in range(2)]
            for i in range(2):
                nc.sync.dma_start(out=b2[i][:], in_=b2_d[i * 128:(i + 1) * 128, :])

            h = ap.tile([128, N], F32, tag="act")
            nc.sync.dma_start(out=h[:], in_=feat_d[:])

            def gcn_gather_layer(h_in, w_rhs, bias, relu, out_tag):
                """v2 layer: node-tile-stationary matmul -> nm scale -> DRAM ->
                gather-sum -> transpose -> ACT epilogue -> feature-major out."""
                hw_t = drp.tile([N, 128], F32, tag="hwtbl")
                # matmul + norm_out scale + store node-major rows to DRAM
                for t in range(NT):
                    psA = pp1.tile([128, 128], F32, space="PSUM", tag="psA")
                    nc.tensor.matmul(out=psA[:], lhsT=h_in[:, t * 128:(t + 1) * 128],
                                     rhs=w_rhs[:], start=True, stop=True)
                    hw_nm = nmp.tile([128, 128], F32, tag="hw_nm")
                    nc.scalar.activation(out=hw_nm[:], in_=psA[:], func=AF.Copy,
                                         scale=nout[:, t:t + 1])
                    nc.sync.dma_start(out=hw_t[t * 128:(t + 1) * 128, :], in_=hw_nm[:])
                # gather + 5-sum + transpose + epilogue
                h_out = ap.tile([128, N], F32, tag=out_tag)
                for t in range(NT):
                    gath = gp.tile([128, K, 128], F32, tag="gath")
                    for k in range(K):
                        nc.gpsimd.indirect_dma_start(
                            out=gath[:, k, :], out_offset=None, in_=hw_t[:],
                            in_offset=bass.IndirectOffsetOnAxis(ap=gidx[:, t * K + k:t * K + k + 1], axis=0))
                    agg = nmp.tile([128, 128], F32, tag="agg")
                    nc.vector.tensor_reduce(
                        out=agg[:], in_=gath[:].rearrange("p k f -> p f k"),
                        axis=mybir.AxisListType.X, op=mybir.AluOpType.add)
                    psT = pp2.tile([128, 128], F32, space="PSUM", tag="psT")
                    nc.tensor.transpose(out=psT[:], in_=agg[:], identity=ident[:])
                    nc.scalar.activation(out=h_out[:, t * 128:(t + 1) * 128], in_=psT[:],
                                         func=AF.Relu if relu else AF.Copy,
                                         bias=bias[:, :1] if bias is not None else 0.0)
                return h_out

            h1 = gcn_gather_layer(h, g1w, g1b, True, "act")
            h2 = gcn_gather_layer(h1, g2w, g2b, True, "act")

            # gc3: aggregate-first. scale+transpose h2 -> node-major DRAM
            hw_t = drp.tile([N, 128], F32, tag="hwtbl")
            for t in range(NT):
                psT = pp2.tile([128, 128], F32, space="PSUM", tag="psT")
                nc.tensor.transpose(out=psT[:], in_=h2[:, t * 128:(t + 1) * 128], identity=ident[:])
                tmp_nm = nmp.tile([128, 128], F32, tag="hw_nm")
                nc.scalar.activation(out=tmp_nm[:], in_=psT[:], func=AF.Copy,
                                     scale=nout[:, t:t + 1])
                nc.sync.dma_start(out=hw_t[t * 128:(t + 1) * 128, :], in_=tmp_nm[:])
            agg3 = ap.tile([128, N], F32, tag="act")
            for t in range(NT):
                gath = gp.tile([128, K, 128], F32, tag="gath")
                for k in range(K):
                    nc.gpsimd.indirect_dma_start(
                        out=gath[:, k, :], out_offset=None, in_=hw_t[:],
                        in_offset=bass.IndirectOffsetOnAxis(ap=gidx[:, t * K + k:t * K + k + 1], axis=0))
                agg = nmp.tile([128, 128], F32, tag="agg")
                nc.vector.tensor_reduce(
                    out=agg[:], in_=gath[:].rearrange("p k f -> p f k"),
                    axis=mybir.AxisListType.X, op=mybir.AluOpType.add)
                psT = pp2.tile([128, 128], F32, space="PSUM", tag="psT")
                nc.tensor.transpose(out=psT[:], in_=agg[:], identity=ident[:])
                nc.scalar.activation(out=agg3[:, t * 128:(t + 1) * 128], in_=psT[:], func=AF.Copy)
            # g3 matmul: [256, N] feature-major, weights-stationary (2 col-blocks)
            g3 = [ap.tile([128, N], F32, tag="g3", name=f"g3_{i}") for i in range(2)]
            for half in range(2):
                for j in range(N // 512):
                    ps = pp1.tile([128, 512], F32, space="PSUM", tag="psB")
                    nc.tensor.matmul(out=ps[:], lhsT=g3w[:, half * 128:(half + 1) * 128],
                                     rhs=agg3[:, j * 512:(j + 1) * 512], start=True, stop=True)
                    nc.scalar.activation(out=g3[half][:, j * 512:(j + 1) * 512], in_=ps[:], func=AF.Copy)

            # head, streamed in 512-col chunks
            poolacc = cp.tile([128, 2], F32, tag="poolacc")
            first = [True, True]
            for j in range(N // 512):
                z1 = [hp.tile([128, 512], F32, tag="z1", name=f"z1_{i}") for i in range(4)]
                for mi in range(4):
                    ps = pp1.tile([128, 512], F32, space="PSUM", tag="psB")
                    for ki in range(2):
                        nc.tensor.matmul(out=ps[:], lhsT=w1t[ki][:, mi * 128:(mi + 1) * 128],
                                         rhs=g3[ki][:, j * 512:(j + 1) * 512],
                                         start=(ki == 0), stop=(ki == 1))
                    nc.scalar.activation(out=z1[mi][:], in_=ps[:], func=AF.Relu, bias=b1[mi][:, :1])
                for mi in range(2):
                    ps = pp1.tile([128, 512], F32, space="PSUM", tag="psB")
                    for ki in range(4):
                        nc.tensor.matmul(out=ps[:], lhsT=w2t[ki][:, mi * 128:(mi + 1) * 128],
                                         rhs=z1[ki][:, :], start=(ki == 0), stop=(ki == 3))
                    z2 = hp.tile([128, 512], F32, tag="z2")
                    nc.scalar.activation(out=z2[:], in_=ps[:], func=AF.Relu, bias=b2[mi][:, :1])
                    cmax = hp.tile([128, 1], F32, tag="cmax")
                    nc.vector.tensor_reduce(out=cmax[:], in_=z2[:], axis=mybir.AxisListType.X,
                                            op=mybir.AluOpType.max)
                    if first[mi]:
                        nc.vector.tensor_copy(out=poolacc[:, mi:mi + 1], in_=cmax[:])
                        first[mi] = False
                    else:
                        nc.vector.tensor_tensor(out=poolacc[:, mi:mi + 1], in0=poolacc[:, mi:mi + 1],
                                                in1=cmax[:], op=mybir.AluOpType.max)
            nc.sync.dma_start(out=pooled_d[0:128, :], in_=poolacc[:, 0:1])
            nc.sync.dma_start(out=pooled_d[128:256, :], in_=poolacc[:, 1:2])
    nc.finalize()
    return nc


def _make_runner(nc, n_cores=8):
    import jax
    from jax.sharding import Mesh, PartitionSpec
    from jax.experimental.shard_map import shard_map
    from concourse import bass2jax

    partition_name = nc.partition_id_tensor.name
    in_names, out_names, out_avals, zero_outs = [], [], [], []
    for alloc in nc.m.functions[0].allocations:
        if not isinstance(alloc, mybir.MemoryLocationSet):
            continue
        name = alloc.memorylocations[0].name
        if alloc.kind == "ExternalInput":
            if name != partition_name:
                in_names.append(name)
        elif alloc.kind == "ExternalOutput":
            shape = tuple(alloc.tensor_shape)
            dtype = dt.np(alloc.dtype)
            out_names.append(name)
            out_avals.append(jax.core.ShapedArray(shape, dtype))
            zero_outs.append(np.zeros(shape, dtype))
    n_params = len(in_names)
    all_in_names = in_names + out_names + [partition_name]

    def _body(*args):
        operands = list(args) + [bass2jax.partition_id_tensor()]
        outs = bass2jax._bass_exec_p.bind(
            *operands, out_avals=tuple(out_avals), in_names=tuple(all_in_names),
            out_names=tuple(out_names), lowering_input_output_aliases=(),
            sim_require_finite=True, sim_require_nnan=True, nc=nc)
        return tuple(outs)

    mesh = Mesh(np.asarray(jax.devices()[:n_cores]), ("core",))
    donate = tuple(range(n_params, n_params + len(out_names)))
    sharded = jax.jit(
        shard_map(_body, mesh=mesh,
                  in_specs=(PartitionSpec("core"),) * (n_params + len(out_names)),
                  out_specs=(PartitionSpec("core"),) * len(out_names), check_rep=False),
        donate_argnums=donate, keep_unused=True)

    def run(in_maps, timeit=0):
        import jax as _jax
        concat_in = [np.concatenate([np.asarray(in_maps[c][n]) for c in range(n_cores)], axis=0)
                     for n in in_names]

        def one_call():
            cz = [np.zeros((n_cores * z.shape[0], *z.shape[1:]), z.dtype) for z in zero_outs]
            return _jax.block_until_ready(sharded(*concat_in, *cz))

        outs = one_call()
        results = [{n: np.asarray(outs[i]).reshape(n_cores, *out_avals[i].shape)[c]
                    for i, n in enumerate(out_names)} for c in range(n_cores)]
        tmin = None
        if timeit:
            times = []
            for _ in range(timeit):
                t0 = time.perf_counter()
                one_call()
                times.append(time.perf_counter() - t0)
            tmin = min(times)
        return results, tmin
    return run


def _build_null():
    nc = bacc.Bacc(name="null")
    x_d = nc.dram_tensor("x", [128, 8], F32, kind="ExternalInput")
    y_d = nc.dram_tensor("y", [128, 8], F32, kind="ExternalOutput")
    with TileContext(nc) as tc:
        with tc.tile_pool(name="p", bufs=1) as p:
            t = p.tile([128, 8], F32)
            nc.sync.dma_start(out=t[:], in_=x_d[:])
            nc.sync.dma_start(out=y_d[:], in_=t[:])
    nc.finalize()
    return nc


_CACHE = {}


def _get_programs():
    if "l1" not in _CACHE:
        _CACHE["l1"] = _make_runner(_build_l1())
        _CACHE["l2"] = _make_runner(_build_l2())
    return _CACHE["l1"], _CACHE["l2"]


def kernel(pointcloud, conv_w1, bn1_g, bn1_b, conv_w2, bn2_g, bn2_b,
           conv_w3, bn3_g, bn3_b, conv_w4, bn4_g, bn4_b,
           gc1_w, gc1_b, gc2_w, gc2_b, gc3_w, gc3_b,
           fc1_w, bnf1_g, bnf1_b, fc2_w, bnf2_g, bnf2_b, fc3_w, fc3_b):
    f32 = np.float32
    pts = np.asarray(pointcloud, f32)[..., 0]              # [B, 3, N]
    centroid = pts.mean(axis=2, keepdims=True, dtype=f32)
    d_cent = ((pts - centroid) ** 2).sum(axis=1, dtype=f32)
    order = np.argsort(d_cent, axis=-1, kind="stable")
    xs = np.take_along_axis(pts, order[:, None, :], axis=2)  # [B, 3, N]

    sq = (pts ** 2).sum(axis=1, dtype=f32)                  # [B, N]
    lhs5 = np.zeros((B, 5, N), f32)
    lhs5[:, 0:3] = pts
    lhs5[:, 3] = -sq
    lhs5[:, 4] = -1.0
    rhs5 = np.zeros((B, 5, N), f32)
    rhs5[:, 0:3] = 2.0 * pts
    rhs5[:, 3] = 1.0
    rhs5[:, 4] = sq

    def effw(w, g):
        return (np.asarray(g, f32)[:, None] * np.asarray(w, f32)).T.copy()  # [fi, fo]

    w1, w2 = effw(conv_w1, bn1_g), effw(conv_w2, bn2_g)
    w3, w4 = effw(conv_w3, bn3_g), effw(conv_w4, bn4_g)
    b14 = [np.asarray(b, f32).reshape(-1, 1) for b in (bn1_b, bn2_b, bn3_b, bn4_b)]

    run1, run2 = _get_programs()
    ins1 = [{
        "xs": np.ascontiguousarray(xs[c]), "lhs5": np.ascontiguousarray(lhs5[c]),
        "rhs5": np.ascontiguousarray(rhs5[c]),
        "w1": w1, "w2": w2, "w3": w3, "w4": w4,
        "b1": b14[0], "b2": b14[1], "b3": b14[2], "b4": b14[3],
    } for c in range(B)]
    _CACHE["ins1"] = ins1
    res1, _ = run1(ins1)

    nrm5 = f32(1.0) / np.sqrt(f32(5.0))
    g1w = np.asarray(gc1_w, f32) * nrm5                    # [128, 128] fi x fo
    g2w = np.asarray(gc2_w, f32) * nrm5
    g3w = np.asarray(gc3_w, f32) * nrm5                    # [128, 256]
    w1_eff = np.asarray(bnf1_g, f32)[:, None] * np.asarray(fc1_w, f32)   # [512, 256]
    b1_eff = (np.asarray(bnf1_b, f32)
              + np.asarray(bnf1_g, f32) * (np.asarray(fc1_w, f32) @ np.asarray(gc3_b, f32)))
    w2_eff = np.asarray(bnf2_g, f32)[:, None] * np.asarray(fc2_w, f32)   # [256, 512]
    b2_eff = np.asarray(bnf2_b, f32)

    ins2 = []
    for c in range(B):
        nbr8 = res1[c]["nbr"]                      # [N, 8] uint32
        nbr5 = nbr8[:, :K].astype(np.int64)
        deg = np.bincount(nbr5.reshape(-1), minlength=N).astype(f32)
        norm_out = (f32(1.0) / np.sqrt(np.maximum(deg, f32(1.0)))).astype(f32)
        gidx = nbr5.reshape(NT, 128, K).transpose(1, 0, 2).reshape(128, -1).astype(np.int32).copy()
        ins2.append({
            "feat": res1[c]["feat"],
            "ident": np.eye(128, dtype=f32),
            "gidx": gidx,
            "nout": np.ascontiguousarray(norm_out.reshape(NT, 128).T),
            "g1w": g1w, "g2w": g2w, "g3w": g3w,
            "g1b": np.asarray(gc1_b, f32).reshape(-1, 1),
            "g2b": np.asarray(gc2_b, f32).reshape(-1, 1),
            "w1t": np.ascontiguousarray(w1_eff.T), "b1": b1_eff.reshape(-1, 1),
            "w2t": np.ascontiguousarray(w2_eff.T), "b2": b2_eff.reshape(-1, 1),
        })
    _CACHE["ins2"] = ins2
    res2, _ = run2(ins2)

    pooled = np.max(np.stack([res2[c]["pooled"][:, 0] for c in range(B)]), axis=0)
    out = np.asarray(fc3_w, f32) @ pooled + np.asarray(fc3_b, f32)
    return out.astype(f32)


def measure_hw_ns(reps=30):
    """Estimate device exec time (ns) of L1+L2 via wall-min minus null overhead.

    Must be called after kernel() so the phase inputs are cached.
    """
    run1, run2 = _get_programs()
    if "null" not in _CACHE:
        _CACHE["null"] = _make_runner(_build_null())
    nullrun = _CACHE["null"]
    nins = [{"x": np.zeros((128, 8), np.float32)}] * 8
    _, t0 = nullrun(nins, timeit=reps)
    _, t1 = run1(_CACHE["ins1"], timeit=reps)
    _, t2 = run2(_CACHE["ins2"], timeit=reps)
    l1_ns = max(0.0, (t1 - t0)) * 1e9
    l2_ns = max(0.0, (t2 - t0)) * 1e9
    return l1_ns, l2_ns, t0 * 1e9
